# revision 42
# baseline (speedup 1.0000x reference)
"""Trainium2 Bass kernel for a quantized-conv BasicBlock.

  out = relu(BN2(conv3x3(relu(BN1(conv3x3(x, q(w1)))), q(w2))) + x)

Strategy: data-parallel over batch across 8 cores (4 images each).
BatchNorm statistics are global over the batch; each core computes
per-channel partial sums (sum, sumsq) of the unscaled integer conv
output, AllGathers the [128,2] partials (shared-output fast path,
cheaper than the Mesh AllReduce for tiny payloads) and reduces the 8
shards locally.

Conv mapping: channels (128) on SBUF partitions; the 3x3 pad=1 conv is
9 PSUM-accumulated matmuls per 8-row chunk (N=448), issued tap-major
over groups of 2-4 chunks so one LDWEIGHTS serves several matmuls.
All matmul operands are bf16 (quantized weights are integers in
{-4..3}, exact in bf16); accumulation is fp32 in PSUM.
"""

import os
import numpy as np

N_CORES = 8
B, C, H, W = 32, 128, 56, 56
BL = B // N_CORES            # images per core
HP, WP = H + 2, W + 2        # padded image dims
PIX = H * W                  # 3136
PPIX = HP * WP               # 3364
RC = 8                       # output rows per PSUM chunk
NCHUNK = H // RC             # 7 chunks per image
NTOT = float(B * H * W)      # BN reduction size
BN_EPS = 1e-5
QN, QP = -4.0, 3.0           # 3-bit LSQ range

LAST_RESULTS = None          # BassKernelResults of the most recent run


def _quantize_int(w: np.ndarray, alpha: np.ndarray):
    """Replicate the reference LSQ forward math in fp32; return the
    integer-valued quantized weights (round(clip(w/alpha_s))) and alpha_s."""
    w = np.asarray(w, dtype=np.float32)
    alpha = np.float32(np.asarray(alpha, dtype=np.float32).reshape(-1)[0])
    g = np.float32(1.0) / np.sqrt(np.float32(w.size * 3.0))
    ag = np.float32(alpha * g)
    alpha_s = np.float32(ag + np.float32(alpha - ag))
    with np.errstate(divide="ignore", invalid="ignore"):
        wc = np.clip((w / alpha_s).astype(np.float32), np.float32(QN), np.float32(QP))
    wq = np.rint(wc).astype(np.float32)
    return wq, alpha_s


def _build_program(as1: float, as2: float):
    import concourse.bacc as bacc
    import concourse.tile as tile
    import concourse.mybir as mybir

    f32 = mybir.dt.float32
    bf16 = mybir.dt.bfloat16
    AF = mybir.ActivationFunctionType
    ALU = mybir.AluOpType
    AX = mybir.AxisListType

    nc = bacc.Bacc("TRN2", target_bir_lowering=False, debug=False,
                   num_devices=N_CORES)

    f8 = mybir.dt.float8e4
    DRM = mybir.MatmulPerfMode.DoubleRow
    xp_d = nc.dram_tensor("xp", [BL, C, HP * 128], f8, kind="ExternalInput")
    xi_d = nc.dram_tensor("xi", [BL, C, PIX], bf16, kind="ExternalInput")
    w1_d = nc.dram_tensor("w1t", [C, 9, 2, C], f8, kind="ExternalInput")
    w2_d = nc.dram_tensor("w2t", [C, 9, C], bf16, kind="ExternalInput")
    ga1_d = nc.dram_tensor("ga1", [C, 1], f32, kind="ExternalInput")
    be1_d = nc.dram_tensor("be1", [C, 1], f32, kind="ExternalInput")
    ga2_d = nc.dram_tensor("ga2", [C, 1], f32, kind="ExternalInput")
    be2_d = nc.dram_tensor("be2", [C, 1], f32, kind="ExternalInput")
    eye_d = nc.dram_tensor("eye", [C, C], f32, kind="ExternalInput")
    y_d = nc.dram_tensor("y", [BL, C, PIX], bf16, kind="ExternalOutput")

    groups = [list(range(N_CORES))]
    R0A = 19                 # image-0 first DMA piece: padded rows [0, 19)

    with tile.TileContext(nc) as tc:
        with (
            tc.tile_pool(name="persist", bufs=1) as persist,
            tc.tile_pool(name="xp_p", bufs=BL) as xp_p,
            tc.tile_pool(name="xi_p", bufs=BL) as xi_p,
            tc.tile_pool(name="a1_p", bufs=BL) as a1_p,
            tc.tile_pool(name="o2_p", bufs=BL) as o2_p,
            tc.tile_pool(name="scr_p", bufs=2) as scr_p,
            tc.tile_pool(name="psum", bufs=8, space="PSUM") as psum_p,
            tc.tile_pool(name="dram", bufs=4, space="DRAM") as dram_p,
        ):
            # ---- weights / BN params / input images ----------------------
            w1_t = persist.tile([C, 9, 2, C], f8, tag="w1", name="w1")
            w2_t = persist.tile([C, 9, C], bf16, tag="w2", name="w2")
            ga1 = persist.tile([C, 1], f32, tag="ga1", name="ga1")
            be1 = persist.tile([C, 1], f32, tag="be1", name="be1")
            ga2 = persist.tile([C, 1], f32, tag="ga2", name="ga2")
            be2 = persist.tile([C, 1], f32, tag="be2", name="be2")

            # conv1 input: fp8 (hi, lo) row-planes padded to 64 cols so the
            # DoubleRow ifmap k-pair sits at stride 64 (needs %16==0)
            xp_t = [xp_p.tile([C, HP, 2, 64], f8, tag="xp", name=f"xp{b}")
                    for b in range(BL)]
            xi_t = [xi_p.tile([C, H, W], bf16, tag="xi", name=f"xi{b}")
                    for b in range(BL)]

            # warm-up collectives: the first CC op in a NEFF pays tens of us
            # of staging + rank-skew sync; run two dummy AllGathers (junk
            # data, no dependencies) as the very first gpsimd work so the
            # BN1 gather hits a warm, rank-synced stream.
            wci = dram_p.tile([C, 2], f32, tag="wci", name="wci")
            wco = dram_p.tile([N_CORES, C, 2], f32, tag="wco", name="wco",
                              addr_space="Shared")
            wco2 = dram_p.tile([N_CORES, C, 2], f32, tag="wco2", name="wco2",
                               addr_space="Shared")
            nc.gpsimd.collective_compute(
                "AllGather", ALU.bypass, replica_groups=groups,
                ins=[wci.opt()], outs=[wco.opt()],
            )
            nc.gpsimd.collective_compute(
                "AllGather", ALU.bypass, replica_groups=groups,
                ins=[wci.opt()], outs=[wco2.opt()],
            )

            # w1's first taps and the first rows of image 0 gate the first
            # matmul; put them first on separate queues, split fine.  xi
            # (residual copies) are only needed at the very end - load last.
            nc.sync.dma_start(w1_t[:, 0:2, :, :], w1_d.ap()[:, 0:2, :, :])
            nc.scalar.dma_start(xp_t[0][:, 0:R0A, :, :],
                                xp_d.ap()[0][:, 0:R0A * 128])
            nc.sync.dma_start(w1_t[:, 2:9, :, :], w1_d.ap()[:, 2:9, :, :])
            nc.sync.dma_start(xp_t[0][:, R0A:HP, :, :],
                              xp_d.ap()[0][:, R0A * 128:])
            nc.scalar.dma_start(xp_t[1][:], xp_d.ap()[1])
            nc.sync.dma_start(xp_t[2][:], xp_d.ap()[2])
            nc.scalar.dma_start(xp_t[3][:], xp_d.ap()[3])
            nc.sync.dma_start(w2_t[:], w2_d.ap())
            nc.gpsimd.dma_start(ga1[:], ga1_d.ap())
            nc.gpsimd.dma_start(be1[:], be1_d.ap())
            nc.gpsimd.dma_start(ga2[:], ga2_d.ap())
            nc.gpsimd.dma_start(be2[:], be2_d.ap())
            eye_t = persist.tile([C, C], f32, tag="eye", name="eye")
            nc.gpsimd.dma_start(eye_t[:], eye_d.ap())
            for b in range(BL):
                (nc.sync if b % 2 == 0 else nc.scalar).dma_start(
                    xi_t[b][:], xi_d.ap()[b])

            # ---- per-image persistent buffers ----------------------------
            a1_t, o2_t = [], []
            for b in range(BL):
                at = a1_p.tile([C, HP, WP], bf16, tag="a1", name=f"a1_{b}")
                # zero the 1-pixel border once; interior is fully overwritten
                nc.vector.memset(at[:, 0, :], 0.0)
                nc.vector.memset(at[:, HP - 1, :], 0.0)
                nc.vector.memset(at[:, 1:HP - 1, 0], 0.0)
                nc.vector.memset(at[:, 1:HP - 1, WP - 1], 0.0)
                a1_t.append(at)
                o2_t.append(o2_p.tile([C, H, W], bf16, tag="o2", name=f"o2_{b}"))


            # partial-stat columns: one col per (image, chunk)
            s1a = persist.tile([C, BL * NCHUNK], f32, tag="s1a", name="s1a")
            s2a = persist.tile([C, BL * NCHUNK], f32, tag="s2a", name="s2a")
            s1b = persist.tile([C, BL * NCHUNK], f32, tag="s1b", name="s1b")
            s2b = persist.tile([C, BL * NCHUNK], f32, tag="s2b", name="s2b")

            def conv(src_tiles, w_t, dst, s1cols, s2cols, dr=False):
                """3x3 conv of all images, issued tap-major over groups of
                chunks so one weight load serves several matmuls.  Image 0
                uses finer groups so its first chunks (gated on the input
                DMA for conv1 / the BN1 apply for conv2) start earlier.
                dr=True: fp8 DoubleRow - the source holds (hi, lo) fp8
                interleaved in the innermost dim; each tap is one K=256
                matmul at 2 MACs/cycle, with the tap weight replicated on
                both k-halves so hi+lo accumulates exactly."""
                for b in range(BL):
                    src = src_tiles[b]
                    gspec = ((0, 2), (2, 4), (4, NCHUNK)) if b == 0 else \
                            ((0, 4), (4, NCHUNK))
                    for (cs, ce) in gspec:
                        pss = [psum_p.tile([C, RC, W], f32, tag="ps",
                                           name=f"ps_{b}_{c}")
                               for c in range(cs, ce)]
                        for t in range(9):
                            kh, kw = t // 3, t % 3
                            for i, c in enumerate(range(cs, ce)):
                                r0 = c * RC
                                if dr:
                                    rhs = src[:, r0 + kh:r0 + kh + RC,
                                              :, kw:kw + W].transpose(
                                                  [0, 2, 1, 3])
                                    nc.tensor.matmul(
                                        pss[i][:], w_t[:, t, :, :], rhs,
                                        start=(t == 0), stop=(t == 8),
                                        perf_mode=DRM,
                                    )
                                else:
                                    rhs = src[:, r0 + kh:r0 + kh + RC,
                                              kw:kw + W]
                                    nc.tensor.matmul(
                                        pss[i][:], w_t[:, t, :], rhs,
                                        start=(t == 0), stop=(t == 8),
                                    )
                        for i, c in enumerate(range(cs, ce)):
                            idx = b * NCHUNK + c
                            scr = scr_p.tile([C, RC, W], f32, tag="scr",
                                             name=f"scr_{b}_{c}")
                            nc.scalar.activation(
                                scr[:], pss[i][:], AF.Square,
                                accum_out=s2cols[:, idx:idx + 1],
                            )
                            nc.vector.tensor_scalar(
                                out=dst(b, c), in0=pss[i][:],
                                scalar1=0.0, scalar2=0.0, op0=ALU.add,
                                op1=ALU.add,
                                accum_out=s1cols[:, idx:idx + 1],
                            )

            def bn_params(s1cols, s2cols, gam, bet, alpha_s, pref):
                """Reduce partials, AllGather (shared output) across cores +
                local reduce, produce per-channel affine (a, b) implementing
                BN on the unscaled conv output."""
                cc = persist.tile([C, 2], f32, tag=pref + "ci", name=pref + "ci")
                nc.vector.tensor_reduce(cc[:, 0:1], s1cols[:], axis=AX.X,
                                        op=ALU.add)
                nc.vector.tensor_reduce(cc[:, 1:2], s2cols[:], axis=AX.X,
                                        op=ALU.add)
                # pack the [128,2] stats into 2 partitions (PE transpose) so
                # the DRAM round-trips are 2 descriptors instead of 128 - the
                # per-lane DMA completion semaphores post ~6us faster.
                tps = psum_p.tile([C, RC * W], f32, tag="ps", name=pref + "tp")
                tp = tps[0:2, 0:C]
                nc.tensor.transpose(tp, cc[:], eye_t[:])
                tcc = persist.tile([2, C], f32, tag=pref + "tc", name=pref + "tc")
                nc.vector.tensor_copy(tcc[:], tp)
                d_in = dram_p.tile([2, C], f32, tag=pref + "di", name=pref + "di")
                d_out = dram_p.tile([N_CORES, 2, C], f32, tag=pref + "do",
                                    name=pref + "do", addr_space="Shared")
                nc.sync.dma_start(d_in[:], tcc[:])
                nc.gpsimd.collective_compute(
                    "AllGather", ALU.bypass, replica_groups=groups,
                    ins=[d_in.opt()], outs=[d_out.opt()],
                )
                rg = persist.tile([2, N_CORES, C], f32, tag=pref + "g",
                                  name=pref + "g")
                nc.sync.dma_start(rg[:], d_out.transpose([1, 0, 2]))

                u1 = persist.tile([2, 4, C], f32, tag=pref + "u1", name=pref + "u1")
                u2 = persist.tile([2, 2, C], f32, tag=pref + "u2", name=pref + "u2")
                u3 = persist.tile([2, C], f32, tag=pref + "u3", name=pref + "u3")
                nc.vector.tensor_add(u1[:], rg[:, 0:4, :], rg[:, 4:8, :])
                nc.vector.tensor_add(u2[:], u1[:, 0:2, :], u1[:, 2:4, :])
                nc.vector.tensor_add(u3[:], u2[:, 0, :], u2[:, 1, :])
                gsts = psum_p.tile([C, RC * W], f32, tag="ps", name=pref + "gp")
                gstp = gsts[:, 0:2]
                nc.tensor.transpose(gstp, u3[:], eye_t[0:2, 0:2])
                gst = persist.tile([C, 2], f32, tag=pref + "gs", name=pref + "gs")
                nc.vector.tensor_copy(gst[:], gstp)

                mu = persist.tile([C, 1], f32, tag=pref + "mu", name=pref + "mu")
                e2 = persist.tile([C, 1], f32, tag=pref + "e2", name=pref + "e2")
                va = persist.tile([C, 1], f32, tag=pref + "va", name=pref + "va")
                rs = persist.tile([C, 1], f32, tag=pref + "rs", name=pref + "rs")
                a_ = persist.tile([C, 1], f32, tag=pref + "a", name=pref + "a")
                b_ = persist.tile([C, 1], f32, tag=pref + "b", name=pref + "b")
                inv_n = float(1.0 / NTOT)
                nc.vector.tensor_scalar_mul(mu[:], gst[:, 0:1], inv_n)
                nc.vector.tensor_scalar_mul(e2[:], gst[:, 1:2], inv_n)
                nc.vector.tensor_mul(va[:], mu[:], mu[:])
                nc.vector.tensor_sub(va[:], e2[:], va[:])
                # var_true + eps = alpha_s^2 * var_int + eps
                nc.vector.tensor_scalar(out=va[:], in0=va[:],
                                        scalar1=float(alpha_s ** 2),
                                        scalar2=BN_EPS,
                                        op0=ALU.mult, op1=ALU.add)
                nc.vector.reciprocal(rs[:], va[:])
                nc.scalar.activation(rs[:], rs[:], AF.Sqrt)
                # a = gamma * alpha_s * rstd ; b = beta - mu_int * a * alpha_s
                # (gam already folded with alpha_s on host: gam = gamma*alpha_s)
                nc.vector.tensor_mul(a_[:], gam[:], rs[:])
                nc.vector.tensor_mul(b_[:], mu[:], a_[:])
                nc.vector.tensor_sub(b_[:], bet[:], b_[:])
                return a_, b_

            # ================= conv1 =====================================
            conv(xp_t, w1_t,
                 lambda b, c: a1_t[b][:, 1 + c * RC:1 + c * RC + RC, 1:1 + W],
                 s1a, s2a, dr=True)

            a1c, b1c = bn_params(s1a, s2a, ga1, be1, as1, "p")

            # BN1 + relu in place on the act1 interior; bands align with the
            # conv2 chunk groups so each group starts as soon as possible.
            for b in range(BL):
                for (lo, hi) in ((0, 17), (17, 33), (33, 56)):
                    iv = a1_t[b][:, 1 + lo:1 + hi, 1:1 + W]
                    nc.scalar.activation(iv, iv, AF.Relu,
                                         bias=b1c[:], scale=a1c[:])

            # ================= conv2 =====================================
            conv(a1_t, w2_t,
                 lambda b, c: o2_t[b][:, c * RC:c * RC + RC, :],
                 s1b, s2b)

            a2c, b2c = bn_params(s1b, s2b, ga2, be2, as2, "q")

            # final: y = relu((a2*z2 + b2) + x) in three elementwise passes,
            # all contiguous bf16 so the DVE runs its 2x packed mode, spread
            # across vector/scalar/gpsimd so no single engine serializes.
            for b in range(BL):
                for hi, (r0, r1) in enumerate(((0, H // 2), (H // 2, H))):
                    idx = 2 * b + hi
                    u = o2_t[b][:, r0:r1, :]
                    xiw = xi_t[b][:, r0:r1, :]
                    if idx in (1, 4, 6):
                        nc.scalar.activation(u, u, AF.Identity,
                                             bias=b2c[:], scale=a2c[:])
                    else:
                        nc.vector.tensor_scalar(
                            out=u, in0=u, scalar1=a2c[:], scalar2=b2c[:],
                            op0=ALU.mult, op1=ALU.add)
                    nc.vector.tensor_tensor(out=u, in0=u, in1=xiw, op=ALU.add)
                    if idx in (0, 2, 3, 5, 7):
                        nc.scalar.activation(u, u, AF.Relu)
                    else:
                        nc.vector.tensor_scalar_max(u, u, 0.0)
                    nc.sync.dma_start(
                        y_d.ap()[b][:, r0 * W:r1 * W], u)

    nc.compile()
    return nc


def _prep_inputs(x, w1, alpha1, gamma1, beta1, w2, alpha2, gamma2, beta2):
    import ml_dtypes
    import concourse.mybir as mybir
    bf16 = ml_dtypes.bfloat16
    f8 = mybir.dt.np(mybir.dt.float8e4)

    x = np.asarray(x, dtype=np.float32)
    wq1, as1 = _quantize_int(np.asarray(w1), np.asarray(alpha1))
    wq2, as2 = _quantize_int(np.asarray(w2), np.asarray(alpha2))

    # [cout, cin, kh, kw] -> [cin, tap, cout] so lhsT slices are [K=cin, M=cout].
    # conv1 runs fp8 DoubleRow with (hi, lo) input halves sharing the tap
    # weight, so w1 is [cin, tap, 2, cout] with the weight replicated.
    w1f = np.ascontiguousarray(
        wq1.reshape(C, C, 9).transpose(1, 2, 0)).astype(f8)
    w1t = np.ascontiguousarray(
        np.broadcast_to(w1f[:, :, None, :], (C, 9, 2, C))).astype(f8)
    w2t = np.ascontiguousarray(
        wq2.reshape(C, C, 9).transpose(1, 2, 0)).astype(bf16)

    ga1 = (np.asarray(gamma1, np.float32) * as1).reshape(C, 1)
    ga2 = (np.asarray(gamma2, np.float32) * as2).reshape(C, 1)
    be1 = np.asarray(beta1, np.float32).reshape(C, 1).copy()
    be2 = np.asarray(beta2, np.float32).reshape(C, 1).copy()

    xb = x.astype(bf16)
    # hi/lo fp8 split: x ~= hi + lo exactly to ~1e-3; interleave so each
    # (hi, lo) pair is adjacent in memory for the DoubleRow ifmap stream.
    xhi = x.astype(f8)
    xlo = (x - xhi.astype(np.float32)).astype(f8)
    xpad = np.zeros((B, C, HP, 2, 64), dtype=f8)
    xpad[:, :, 1:1 + H, 0, 1:1 + W] = xhi
    xpad[:, :, 1:1 + H, 1, 1:1 + W] = xlo

    in_maps = []
    for c in range(N_CORES):
        shard = xpad[c * BL:(c + 1) * BL].reshape(BL, C, HP * 128)
        ishard = xb[c * BL:(c + 1) * BL].reshape(BL, C, PIX)
        in_maps.append({
            "xp": np.ascontiguousarray(shard),
            "xi": np.ascontiguousarray(ishard),
            "w1t": w1t, "w2t": w2t,
            "ga1": ga1, "be1": be1, "ga2": ga2, "be2": be2,
            "eye": np.eye(C, dtype=np.float32),
        })
    return in_maps, float(as1), float(as2)


def kernel(**inputs) -> np.ndarray:
    global LAST_RESULTS
    from concourse.bass_utils import run_bass_kernel_spmd

    in_maps, as1, as2 = _prep_inputs(**inputs)
    nc = _build_program(as1, as2)

    trace = bool(int(os.environ.get("KERNEL_TRACE", "0")))
    res = run_bass_kernel_spmd(
        nc, in_maps, list(range(N_CORES)),
        trace=trace,
    )
    LAST_RESULTS = res
    out = np.stack([np.asarray(res.results[c]["y"]) for c in range(N_CORES)])
    return np.ascontiguousarray(
        out.reshape(B, C, H, W)).astype(np.float32)
